# revision 15
# baseline (speedup 1.0000x reference)
"""Reservoir-computing recurrence for trn2, 8-core tensor-parallel with
time-chunk batching + parareal sweeps.

Key idea: the clamped recurrence contracts perturbations at ~0.7/step, so
split T=2048 into NC=128 chunks of L=16 steps and advance ALL chunk states
simultaneously (they form the 128-wide free dim of real matmuls instead of
matvecs). Run NSWEEP sweeps; between sweeps chunk g's start state is seeded
with chunk g-1's end state (parareal). Error after m sweeps ~ 0.7^(16*m):
48 supersteps replace 2048 sequential steps.

Per-core layout (core c owns W rows [512c, 512c+512)):
  - Superstep u (i = u%L): PE computes P[g, r] = sum_feat F[feat, g] *
    W[512c+r, feat] + x[512c+r, 16g+i] as 2 chains of 33 matmuls
    (lhsT = F k-chunk [128feat, 128g] stationary, rhs = W^T slice [128, 256],
    x injected via identity-stationary matmul), one chain per 256-row half.
  - DVE clamps psum -> clamped[128g, 512r] fp16.
  - PE transposes the four [128g,128r] tiles back to feat-major.
  - ACT copies transpose psum -> hist slot i (fp16) = broadcast source.
  - GPSIMD remote_dma_broadcast sends each 256-row half [128, 256] fp16 to
    all 8 cores' F[(u+1)%2] at column 512*core_id (dynamic via partition_id).
  - Sweep boundary: last step's broadcast lands in Fend; DVE shifts
    Fend cols g -> F[next] cols g+1 (chunk seeding), zeros col g=0.
  - hist keeps the final sweep's states [feat 512(mine), g, i]; readout
    is a dense w_out chunk matmul; host sums the 8 partial outputs and
    unscrambles column order.
"""

import numpy as np
from contextlib import ExitStack

import concourse.bass as bass
import concourse.mybir as mybir
from concourse import library_config

FEAT = 4096
OUT_DIM = 512
N_CORES = 8
SHARD = FEAT // N_CORES          # 512 rows per core
KCH = FEAT // 128                # 32 k-chunks
NC = 128                         # time chunks (= matmul free/stationary dim)
NSWEEP = 2

F16 = mybir.dt.float16
F32 = mybir.dt.float32


def build(T: int, nsweep: int = NSWEEP, no_comm: bool = False, barrier_only: bool = False, fire_forget: bool = False) -> bass.Bass:
    assert T % NC == 0
    L = T // NC                  # steps per sweep (16)
    NS = nsweep * L              # total supersteps
    nc = bass.Bass(target_bir_lowering=False, num_devices=N_CORES)

    # ---- I/O (fp16 pre-permuted on host) ----
    wT_h = nc.dram_tensor("wT", [128, KCH * SHARD], F16, kind="ExternalInput")
    xT_h = nc.dram_tensor("xT", [128, L * SHARD], F16, kind="ExternalInput")
    woT_h = nc.dram_tensor("woT", [128, 4 * OUT_DIM], F16, kind="ExternalInput")
    out_h = nc.dram_tensor("outp", [OUT_DIM, T], F32, kind="ExternalOutput")

    bar_in = nc.inline_tensor(np.zeros((1, 1), np.float32), "bar_in")
    bar_out = nc.dram_tensor("bar_out", [1, 1], F32, addr_space="Shared")
    ident_h = nc.inline_tensor(np.eye(128, dtype=np.float16), "ident")

    FP = NC * KCH                # F free size (4096)
    H0K = [k for k in range(KCH) if k % 4 < 2]    # chunks in row-half 0
    H1K = [k for k in range(KCH) if k % 4 >= 2]

    with ExitStack() as ctx:
        e = ctx.enter_context
        load_sem = e(nc.semaphore("load_sem"))
        pe_sem = e(nc.semaphore("pe_sem"))
        cl_sem = e(nc.semaphore("cl_sem"))      # DVE: clamps, shifts, F0 init
        cp_sem = e(nc.semaphore("cp_sem"))      # ACT: psum->hist copies
        ro_sem = e(nc.semaphore("ro_sem"))      # DVE: readout psum->sbuf
        ls_sem = e(nc.semaphore("ls_sem"))      # local bcast release
        prep_sem = e(nc.semaphore("prep_sem"))  # Q7 desc-gen completion
        rs0a = e(nc.semaphore("rs0a"))
        rs0b = e(nc.semaphore("rs0b"))
        rs1a = e(nc.semaphore("rs1a"))
        rs1b = e(nc.semaphore("rs1b"))
        bar_sem = e(nc.semaphore("bar_sem"))
        odma_sem = e(nc.semaphore("odma_sem"))
        rs0 = [rs0a, rs0b]
        rs1 = [rs1a, rs1b]

        wT_sb = e(nc.sbuf_tensor("wT_sb", [128, KCH * SHARD], F16))
        xT_sb = e(nc.sbuf_tensor("xT_sb", [128, L * SHARD], F16))
        woT_sb = e(nc.sbuf_tensor("woT_sb", [128, 4 * OUT_DIM], F16))
        ident_sb = e(nc.sbuf_tensor("ident_sb", [128, 128], F16))
        F_sb = [e(nc.sbuf_tensor(f"F{b}", [128, FP], F16)) for b in range(2)]
        Fend = e(nc.sbuf_tensor("Fend", [128, FP], F16))
        clamped = [e(nc.sbuf_tensor(f"cl{b}", [128, SHARD], F16)) for b in range(2)]
        hist = e(nc.sbuf_tensor("hist", [128, NS * SHARD], F16))
        outsb = e(nc.sbuf_tensor("outsb", [128, 4 * T], F32))

        pA = [e(nc.psum_tensor(f"pA{b}", [128, 256], F32)) for b in range(2)]
        pB = [e(nc.psum_tensor(f"pB{b}", [128, 256], F32)) for b in range(2)]
        pT = [e(nc.psum_tensor(f"pT{b}", [128, 128], F16)) for b in range(2)]
        pRO = [e(nc.psum_tensor(f"pRO{b}", [128, 512], F32)) for b in range(2)]

        # ---- Python-side semaphore bookkeeping ----
        chain_cnt = [[0, 0] for _ in range(NS)]   # [u][wave] -> pe_sem target
        tr_cnt = [[0] * 4 for _ in range(NS)]     # [u][m]
        clamp_cnt = [[0, 0] for _ in range(NS)]
        shift_cnt = [0] * (nsweep + 1)            # cl_sem value after shift s
        copy_cnt = [[0] * 4 for _ in range(NS)]
        ro_mm_cnt = [0] * 16

        def is_boundary(u):                       # last step of non-final sweep
            return (u % L == L - 1) and (u // L < nsweep - 1)

        sends = [u != NS - 1 for u in range(NS)]  # final step sends nothing

        def rs_through(u):
            # each rs sem gets +16 per sending step of its parity; all steps
            # through u send (u < NS-1 whenever this is used)
            return 16 * (u // 2 + 1)

        # ls value after step u's sends complete
        ls_after = []
        acc = 0
        for u in range(NS):
            if sends[u]:
                acc += 16
            ls_after.append(acc)
        ls_total = acc

        pe_cnt = 0
        for u in range(NS):
            pe_cnt += 1; chain_cnt[u][0] = pe_cnt
            pe_cnt += 1; tr_cnt[u][0] = pe_cnt
            pe_cnt += 1; tr_cnt[u][1] = pe_cnt
            pe_cnt += 1; chain_cnt[u][1] = pe_cnt
            pe_cnt += 1; tr_cnt[u][2] = pe_cnt
            pe_cnt += 1; tr_cnt[u][3] = pe_cnt
        for j in range(16):
            pe_cnt += 1; ro_mm_cnt[j] = pe_cnt

        cl_cnt = 1                                # F0 memset
        for u in range(NS):
            cl_cnt += 1; clamp_cnt[u][0] = cl_cnt
            cl_cnt += 1; clamp_cnt[u][1] = cl_cnt
            if is_boundary(u):
                cl_cnt += 2                       # memset + shift copy
                shift_cnt[u // L + 1] = cl_cnt

        cp_cnt = 0
        for u in range(NS):
            for m in range(4):
                cp_cnt += 1; copy_cnt[u][m] = cp_cnt

        with nc.Block() as block:

            @block.sync
            def _(sp):
                sp.dma_start(wT_sb[:, :], wT_h[:, :]).then_inc(load_sem, 16)
                sp.dma_start(xT_sb[:, :], xT_h[:, :]).then_inc(load_sem, 16)
                sp.dma_start(woT_sb[:, :], woT_h[:, :]).then_inc(load_sem, 16)
                sp.dma_start(ident_sb[:, :], ident_h[:, :]).then_inc(load_sem, 16)
                sp.wait_ge(ro_sem, 16)
                for mo in range(4):
                    sp.dma_start(
                        out_h[128 * mo : 128 * (mo + 1), :],
                        outsb[:, T * mo : T * (mo + 1)],
                    ).then_inc(odma_sem, 16)
                sp.wait_ge(odma_sem, 64)

            @block.tensor
            def _(te):
                te.wait_ge(load_sem, 64)
                te.wait_ge(cl_sem, 1)             # F0 zeroed
                for u in range(NS):
                    par = u % 2
                    i = u % L
                    for w in range(2):
                        pacc = (pA if w == 0 else pB)[par]
                        if u >= 2:                # psum drained by clamp(u-2)
                            te.wait_ge(cl_sem, clamp_cnt[u - 2][w])
                        if w == 0:
                            if u % L == 0 and u > 0:
                                te.wait_ge(cl_sem, shift_cnt[u // L])
                        # x inject (starts accumulation group)
                        te.matmul(
                            pacc[:, :],
                            ident_sb[:, :],
                            xT_sb[:, SHARD * i + 256 * w : SHARD * i + 256 * w + 256],
                            start=True,
                            stop=False,
                        )
                        gated = (u > 0 and u % L != 0) and not (no_comm or barrier_only or fire_forget)
                        if w == 0 and gated:
                            te.wait_ge(rs0[(u - 1) % 2], rs_through(u - 1))
                        for kj, k in enumerate(H0K + H1K):
                            te.matmul(
                                pacc[:, :],
                                F_sb[par][:, 128 * k : 128 * (k + 1)],
                                wT_sb[:, SHARD * k + 256 * w : SHARD * k + 256 * w + 256],
                                start=False,
                                stop=(kj == KCH - 1),
                            ).then_inc(pe_sem, 1) if kj == KCH - 1 else te.matmul(
                                pacc[:, :],
                                F_sb[par][:, 128 * k : 128 * (k + 1)],
                                wT_sb[:, SHARD * k + 256 * w : SHARD * k + 256 * w + 256],
                                start=False,
                                stop=False,
                            )
                        # wave-w transposes right after its chain so the
                        # broadcast of this half overlaps the next chain
                        for m in (2 * w, 2 * w + 1):
                            if m % 2 == 0:
                                te.wait_ge(cl_sem, clamp_cnt[u][w])
                            if m < 2:
                                if u >= 1:
                                    te.wait_ge(cp_sem, copy_cnt[u - 1][m + 2])
                            else:
                                te.wait_ge(cp_sem, copy_cnt[u][m - 2])
                            te.transpose(
                                pT[m % 2][:, :],
                                clamped[par][:, 128 * m : 128 * (m + 1)],
                                ident_sb[:, :],
                            ).then_inc(pe_sem, 1)

                # readout
                te.wait_ge(cp_sem, copy_cnt[NS - 1][3])
                j = 0
                for mo in range(4):
                    for nt in range(4):
                        if j >= 2:
                            te.wait_ge(ro_sem, j - 1)
                        for k in range(4):
                            te.matmul(
                                pRO[j % 2][:, :],
                                woT_sb[:, OUT_DIM * k + 128 * mo : OUT_DIM * k + 128 * (mo + 1)],
                                bass.AP(
                                    hist,
                                    SHARD * (NS - L + 4 * nt) + 128 * k,
                                    [[NS * SHARD, 128], [SHARD, 4], [1, 128]],
                                ),
                                start=(k == 0),
                                stop=(k == 3),
                            ).then_inc(pe_sem, 1) if k == 3 else te.matmul(
                                pRO[j % 2][:, :],
                                woT_sb[:, OUT_DIM * k + 128 * mo : OUT_DIM * k + 128 * (mo + 1)],
                                bass.AP(
                                    hist,
                                    SHARD * (NS - L + 4 * nt) + 128 * k,
                                    [[NS * SHARD, 128], [SHARD, 4], [1, 128]],
                                ),
                                start=(k == 0),
                                stop=False,
                            )
                        j += 1

            @block.vector
            def _(ve):
                ve.memset(F_sb[0][:, :], 0).then_inc(cl_sem, 1)
                for u in range(NS):
                    par = u % 2
                    for w in range(2):
                        ve.wait_ge(pe_sem, chain_cnt[u][w])
                        ve.tensor_scalar(
                            clamped[par][:, 256 * w : 256 * (w + 1)],
                            (pA if w == 0 else pB)[par][:, :],
                            1.0,
                            -1.0,
                            op0=mybir.AluOpType.min,
                            op1=mybir.AluOpType.max,
                        ).then_inc(cl_sem, 1)
                    if is_boundary(u):
                        nxt = (u + 1) % 2
                        if not (no_comm or barrier_only or fire_forget):
                            ve.wait_ge(rs0[par], rs_through(u))
                        ve.memset(
                            bass.AP(F_sb[nxt], 0, [[FP, 128], [NC, KCH], [1, 1]]), 0
                        ).then_inc(cl_sem, 1)
                        ve.tensor_copy(
                            bass.AP(F_sb[nxt], 1, [[FP, 128], [NC, KCH], [1, NC - 1]]),
                            bass.AP(Fend, 0, [[FP, 128], [NC, KCH], [1, NC - 1]]),
                        ).then_inc(cl_sem, 1)
                # readout copies
                j = 0
                for mo in range(4):
                    for nt in range(4):
                        ve.wait_ge(pe_sem, ro_mm_cnt[j])
                        ve.tensor_copy(
                            outsb[:, T * mo + 512 * nt : T * mo + 512 * (nt + 1)],
                            pRO[j % 2][:, :],
                        ).then_inc(ro_sem, 1)
                        j += 1

            @block.scalar
            def _(sc):
                for u in range(NS):
                    # hist has one slot per superstep: no reuse, no DMA
                    # release wait needed before overwriting.
                    for m in range(4):
                        sc.wait_ge(pe_sem, tr_cnt[u][m])
                        sc.activation(
                            hist[:, SHARD * u + 128 * m : SHARD * u + 128 * (m + 1)],
                            pT[m % 2][:, :],
                            mybir.ActivationFunctionType.Copy,
                        ).then_inc(cp_sem, 1)

            @block.gpsimd
            def _(gp):
                if no_comm:
                    return
                gp.load_library(library_config.remote_dma)
                gp.collective_compute(
                    "AllReduce",
                    mybir.AluOpType.add,
                    replica_groups=[list(range(N_CORES))],
                    ins=[bar_in.ap().opt()],
                    outs=[bar_out.ap().opt()],
                ).then_inc(bar_sem, 1)
                gp.wait_ge(bar_sem, 1)

                cid = gp.partition_id()
                with gp.register("r0") as R0, gp.register("r1") as R1:
                    gp.reg_alu(R0, cid, SHARD, op=mybir.AluOpType.mult)
                    gp.reg_alu(R1, R0, 256, op=mybir.AluOpType.add)
                    c0 = gp.snap(R0, min_val=0, max_val=SHARD * 7)
                    c1 = gp.snap(R1, min_val=256, max_val=SHARD * 7 + 256)

                    def prep(u):
                        par = u % 2
                        dst = Fend if is_boundary(u) else F_sb[(u + 1) % 2]
                        gp.remote_dma_broadcast(
                            dst[:, bass.ds(c0, SHARD)],
                            hist[:, SHARD * u : SHARD * u + SHARD],
                            remote_sem=rs0[par],
                            local_sem=ls_sem,
                            rdests=[(0, k) for k in range(N_CORES)],
                        ).then_inc(prep_sem, 1)

                    if not barrier_only:
                        prep(0)
                    for u in range(NS):
                        if barrier_only or not sends[u]:
                            continue
                        gp.wait_ge(prep_sem, u + 1)
                        gp.wait_ge(cp_sem, copy_cnt[u][3])
                        gp.trigger_dma(count=1)
                        if u + 1 < NS and sends[u + 1]:
                            prep(u + 1)
                    if not barrier_only:
                        gp.wait_ge(ls_sem, ls_total)
                gp.collective_compute(
                    "AllReduce",
                    mybir.AluOpType.add,
                    replica_groups=[list(range(N_CORES))],
                    ins=[bar_in.ap().opt()],
                    outs=[bar_out.ap().opt()],
                ).then_inc(bar_sem, 1)
                gp.wait_ge(bar_sem, 2)

    mybir.codegen_inst_isa_subclasses(nc)
    return nc


# ---------------- host-side data prep ----------------

def prep_inputs(W, x, w_out, T):
    """Returns per-core input dicts. Arg order matches the old kernel."""
    W = np.asarray(W, np.float32)
    x = np.asarray(x, np.float32)
    w_out = np.asarray(w_out, np.float32)
    L = T // NC
    WT = np.ascontiguousarray(W.T)                      # [feat_in, row]
    maps = []
    for c in range(N_CORES):
        lo = SHARD * c
        # wT[p, k*512+r] = W[lo+r, 128k+p]
        wT = np.ascontiguousarray(
            WT.reshape(KCH, 128, FEAT)[:, :, lo : lo + SHARD]
            .transpose(1, 0, 2)
            .reshape(128, KCH * SHARD)
        ).astype(np.float16)
        # xT[g, i*512+r] = x[lo+r, L*g+i]
        xT = np.ascontiguousarray(
            x[lo : lo + SHARD, :].reshape(SHARD, NC, L)
            .transpose(1, 2, 0)
            .reshape(NC, L * SHARD)
        ).astype(np.float16)
        # woT[p, k*512 + mo*128 + o] = w_out[128mo+o, lo + 128k + p]
        woT = np.ascontiguousarray(
            w_out[:, lo : lo + SHARD].reshape(4, 128, 4, 128)
            .transpose(3, 2, 0, 1)
            .reshape(128, 4 * OUT_DIM)
        ).astype(np.float16)
        maps.append({"wT": wT, "xT": xT, "woT": woT})
    return maps


from concourse.bass_utils import run_bass_kernel_spmd

_cache = {}


def kernel(x: np.ndarray, W_res: np.ndarray, w_out: np.ndarray) -> np.ndarray:
    T = x.shape[1]
    if "nc" not in _cache or _cache.get("T") != T:
        _cache["nc"] = build(T)
        _cache["T"] = T
    nc = _cache["nc"]
    maps = prep_inputs(np.asarray(W_res), np.asarray(x), np.asarray(w_out), T)
    res = run_bass_kernel_spmd(nc, maps, core_ids=list(range(N_CORES)))
    raw = np.zeros((OUT_DIM, T), np.float32)
    for r in res.results:
        raw += r["outp"]
    # cols are [nt, ii, gg] with t = gg*L + nt*(L//4) + ii; invert
    L = T // NC
    out = raw.reshape(OUT_DIM, 4, L // 4, NC).transpose(0, 3, 1, 2).reshape(OUT_DIM, T)
    return out


# revision 16
# speedup vs baseline: 1.2407x; 1.2407x over previous
"""Reservoir-computing recurrence for trn2, 8-core tensor-parallel with
time-chunk batching + parareal sweeps.

Key idea: the clamped recurrence contracts perturbations at ~0.7/step, so
split T=2048 into NC=128 chunks of L=16 steps and advance ALL chunk states
simultaneously (they form the 128-wide free dim of real matmuls instead of
matvecs). Run NSWEEP sweeps; between sweeps chunk g's start state is seeded
with chunk g-1's end state (parareal). Error after m sweeps ~ 0.7^(16*m):
48 supersteps replace 2048 sequential steps.

Per-core layout (core c owns W rows [512c, 512c+512)):
  - Superstep u (i = u%L): PE computes P[g, r] = sum_feat F[feat, g] *
    W[512c+r, feat] + x[512c+r, 16g+i] as 2 chains of 33 matmuls
    (lhsT = F k-chunk [128feat, 128g] stationary, rhs = W^T slice [128, 256],
    x injected via identity-stationary matmul), one chain per 256-row half.
  - DVE clamps psum -> clamped[128g, 512r] fp16.
  - PE transposes the four [128g,128r] tiles back to feat-major.
  - ACT copies transpose psum -> hist slot i (fp16) = broadcast source.
  - GPSIMD remote_dma_broadcast sends each 256-row half [128, 256] fp16 to
    all 8 cores' F[(u+1)%2] at column 512*core_id (dynamic via partition_id).
  - Sweep boundary: last step's broadcast lands in Fend; DVE shifts
    Fend cols g -> F[next] cols g+1 (chunk seeding), zeros col g=0.
  - hist keeps the final sweep's states [feat 512(mine), g, i]; readout
    is a dense w_out chunk matmul; host sums the 8 partial outputs and
    unscrambles column order.
"""

import numpy as np
from contextlib import ExitStack

import concourse.bass as bass
import concourse.mybir as mybir
from concourse import library_config

FEAT = 4096
OUT_DIM = 512
N_CORES = 8
SHARD = FEAT // N_CORES          # 512 rows per core
KCH = FEAT // 128                # 32 k-chunks
NC = 128                         # time chunks (= matmul free/stationary dim)
NSWEEP = 2

F16 = mybir.dt.float16
F32 = mybir.dt.float32


def build(T: int, nsweep: int = NSWEEP, no_comm: bool = False, barrier_only: bool = False, fire_forget: bool = False) -> bass.Bass:
    assert T % NC == 0
    L = T // NC                  # steps per sweep (16)
    NS = nsweep * L              # total supersteps
    nc = bass.Bass(target_bir_lowering=False, num_devices=N_CORES)

    # ---- I/O (fp16 pre-permuted on host) ----
    wT_h = nc.dram_tensor("wT", [128, KCH * SHARD], F16, kind="ExternalInput")
    xT_h = nc.dram_tensor("xT", [128, L * SHARD], F16, kind="ExternalInput")
    woT_h = nc.dram_tensor("woT", [128, 4 * OUT_DIM], F16, kind="ExternalInput")
    out_h = nc.dram_tensor("outp", [OUT_DIM, T], F32, kind="ExternalOutput")

    bar_in = nc.inline_tensor(np.zeros((1, 1), np.float32), "bar_in")
    bar_out = nc.dram_tensor("bar_out", [1, 1], F32, addr_space="Shared")
    ident_h = nc.inline_tensor(np.eye(128, dtype=np.float16), "ident")

    FP = NC * KCH                # F free size (4096)
    H0K = [k for k in range(KCH) if k % 4 < 2]    # chunks in row-half 0
    H1K = [k for k in range(KCH) if k % 4 >= 2]

    with ExitStack() as ctx:
        e = ctx.enter_context
        load_sem = e(nc.semaphore("load_sem"))
        pe_sem = e(nc.semaphore("pe_sem"))
        cl_sem = e(nc.semaphore("cl_sem"))      # DVE: clamps, shifts, F0 init
        cp_sem = e(nc.semaphore("cp_sem"))      # ACT: psum->hist copies
        ro_sem = e(nc.semaphore("ro_sem"))      # DVE: readout psum->sbuf
        ls_sem = e(nc.semaphore("ls_sem"))      # local bcast release
        prep_sem = e(nc.semaphore("prep_sem"))  # Q7 desc-gen completion
        rs0a = e(nc.semaphore("rs0a"))
        rs0b = e(nc.semaphore("rs0b"))
        rs1a = e(nc.semaphore("rs1a"))
        rs1b = e(nc.semaphore("rs1b"))
        bar_sem = e(nc.semaphore("bar_sem"))
        odma_sem = e(nc.semaphore("odma_sem"))
        rs0 = [rs0a, rs0b]
        rs1 = [rs1a, rs1b]

        wT_sb = e(nc.sbuf_tensor("wT_sb", [128, KCH * SHARD], F16))
        xT_sb = e(nc.sbuf_tensor("xT_sb", [128, L * SHARD], F16))
        woT_sb = e(nc.sbuf_tensor("woT_sb", [128, 4 * OUT_DIM], F16))
        ident_sb = e(nc.sbuf_tensor("ident_sb", [128, 128], F16))
        F_sb = [e(nc.sbuf_tensor(f"F{b}", [128, FP], F16)) for b in range(2)]
        Fend = e(nc.sbuf_tensor("Fend", [128, FP], F16))
        clamped = [e(nc.sbuf_tensor(f"cl{b}", [128, SHARD], F16)) for b in range(2)]
        hist = e(nc.sbuf_tensor("hist", [128, NS * SHARD], F16))
        outsb = e(nc.sbuf_tensor("outsb", [128, 4 * T], F32))

        pA = [e(nc.psum_tensor(f"pA{b}", [128, 256], F32)) for b in range(2)]
        pB = [e(nc.psum_tensor(f"pB{b}", [128, 256], F32)) for b in range(2)]
        pT = [e(nc.psum_tensor(f"pT{b}", [128, 128], F16)) for b in range(2)]
        pRO = [e(nc.psum_tensor(f"pRO{b}", [128, 512], F32)) for b in range(2)]

        # ---- Python-side semaphore bookkeeping ----
        chain_cnt = [[0, 0] for _ in range(NS)]   # [u][wave] -> pe_sem target
        tr_cnt = [[0] * 4 for _ in range(NS)]     # [u][m]
        clamp_cnt = [[0, 0] for _ in range(NS)]
        shift_cnt = [0] * (nsweep + 1)            # cl_sem value after shift s
        copy_cnt = [[0] * 4 for _ in range(NS)]
        ro_mm_cnt = [0] * 16

        def is_boundary(u):                       # last step of non-final sweep
            return (u % L == L - 1) and (u // L < nsweep - 1)

        sends = [u != NS - 1 for u in range(NS)]  # final step sends nothing

        def rs_through(u):
            # each rs sem gets +16 per sending step of its parity; all steps
            # through u send (u < NS-1 whenever this is used)
            return 16 * (u // 2 + 1)

        # ls value after step u's sends complete
        ls_after = []
        acc = 0
        for u in range(NS):
            if sends[u]:
                acc += 32
            ls_after.append(acc)
        ls_total = acc

        pe_cnt = 0
        for u in range(NS):
            pe_cnt += 1; chain_cnt[u][0] = pe_cnt
            pe_cnt += 1; tr_cnt[u][0] = pe_cnt
            pe_cnt += 1; tr_cnt[u][1] = pe_cnt
            pe_cnt += 1; chain_cnt[u][1] = pe_cnt
            pe_cnt += 1; tr_cnt[u][2] = pe_cnt
            pe_cnt += 1; tr_cnt[u][3] = pe_cnt
        for j in range(16):
            pe_cnt += 1; ro_mm_cnt[j] = pe_cnt

        cl_cnt = 1                                # F0 memset
        for u in range(NS):
            cl_cnt += 1; clamp_cnt[u][0] = cl_cnt
            cl_cnt += 1; clamp_cnt[u][1] = cl_cnt
            if is_boundary(u):
                cl_cnt += 2                       # memset + shift copy
                shift_cnt[u // L + 1] = cl_cnt

        cp_cnt = 0
        for u in range(NS):
            for m in range(4):
                cp_cnt += 1; copy_cnt[u][m] = cp_cnt

        with nc.Block() as block:

            @block.sync
            def _(sp):
                sp.dma_start(wT_sb[:, :], wT_h[:, :]).then_inc(load_sem, 16)
                sp.dma_start(xT_sb[:, :], xT_h[:, :]).then_inc(load_sem, 16)
                sp.dma_start(woT_sb[:, :], woT_h[:, :]).then_inc(load_sem, 16)
                sp.dma_start(ident_sb[:, :], ident_h[:, :]).then_inc(load_sem, 16)
                sp.wait_ge(ro_sem, 16)
                for mo in range(4):
                    sp.dma_start(
                        out_h[128 * mo : 128 * (mo + 1), :],
                        outsb[:, T * mo : T * (mo + 1)],
                    ).then_inc(odma_sem, 16)
                sp.wait_ge(odma_sem, 64)

            @block.tensor
            def _(te):
                te.wait_ge(load_sem, 64)
                te.wait_ge(cl_sem, 1)             # F0 zeroed
                for u in range(NS):
                    par = u % 2
                    i = u % L
                    for w in range(2):
                        pacc = (pA if w == 0 else pB)[par]
                        if u >= 2:                # psum drained by clamp(u-2)
                            te.wait_ge(cl_sem, clamp_cnt[u - 2][w])
                        if w == 0:
                            if u % L == 0 and u > 0:
                                te.wait_ge(cl_sem, shift_cnt[u // L])
                        # x inject (starts accumulation group)
                        te.matmul(
                            pacc[:, :],
                            ident_sb[:, :],
                            xT_sb[:, SHARD * i + 256 * w : SHARD * i + 256 * w + 256],
                            start=True,
                            stop=False,
                        )
                        gated = (u > 0 and u % L != 0) and not (no_comm or barrier_only or fire_forget)
                        if w == 0 and gated:
                            te.wait_ge(rs0[(u - 1) % 2], rs_through(u - 1))
                        for kj, k in enumerate(H0K + H1K):
                            if w == 0 and kj == len(H0K) and gated:
                                te.wait_ge(rs1[(u - 1) % 2], rs_through(u - 1))
                            te.matmul(
                                pacc[:, :],
                                F_sb[par][:, 128 * k : 128 * (k + 1)],
                                wT_sb[:, SHARD * k + 256 * w : SHARD * k + 256 * w + 256],
                                start=False,
                                stop=(kj == KCH - 1),
                            ).then_inc(pe_sem, 1) if kj == KCH - 1 else te.matmul(
                                pacc[:, :],
                                F_sb[par][:, 128 * k : 128 * (k + 1)],
                                wT_sb[:, SHARD * k + 256 * w : SHARD * k + 256 * w + 256],
                                start=False,
                                stop=False,
                            )
                        # wave-w transposes right after its chain so the
                        # broadcast of this half overlaps the next chain
                        for m in (2 * w, 2 * w + 1):
                            if m % 2 == 0:
                                te.wait_ge(cl_sem, clamp_cnt[u][w])
                            if m < 2:
                                if u >= 1:
                                    te.wait_ge(cp_sem, copy_cnt[u - 1][m + 2])
                            else:
                                te.wait_ge(cp_sem, copy_cnt[u][m - 2])
                            te.transpose(
                                pT[m % 2][:, :],
                                clamped[par][:, 128 * m : 128 * (m + 1)],
                                ident_sb[:, :],
                            ).then_inc(pe_sem, 1)

                # readout
                te.wait_ge(cp_sem, copy_cnt[NS - 1][3])
                j = 0
                for mo in range(4):
                    for nt in range(4):
                        if j >= 2:
                            te.wait_ge(ro_sem, j - 1)
                        for k in range(4):
                            te.matmul(
                                pRO[j % 2][:, :],
                                woT_sb[:, OUT_DIM * k + 128 * mo : OUT_DIM * k + 128 * (mo + 1)],
                                bass.AP(
                                    hist,
                                    SHARD * (NS - L + 4 * nt) + 128 * k,
                                    [[NS * SHARD, 128], [SHARD, 4], [1, 128]],
                                ),
                                start=(k == 0),
                                stop=(k == 3),
                            ).then_inc(pe_sem, 1) if k == 3 else te.matmul(
                                pRO[j % 2][:, :],
                                woT_sb[:, OUT_DIM * k + 128 * mo : OUT_DIM * k + 128 * (mo + 1)],
                                bass.AP(
                                    hist,
                                    SHARD * (NS - L + 4 * nt) + 128 * k,
                                    [[NS * SHARD, 128], [SHARD, 4], [1, 128]],
                                ),
                                start=(k == 0),
                                stop=False,
                            )
                        j += 1

            @block.vector
            def _(ve):
                ve.memset(F_sb[0][:, :], 0).then_inc(cl_sem, 1)
                for u in range(NS):
                    par = u % 2
                    for w in range(2):
                        ve.wait_ge(pe_sem, chain_cnt[u][w])
                        ve.tensor_scalar(
                            clamped[par][:, 256 * w : 256 * (w + 1)],
                            (pA if w == 0 else pB)[par][:, :],
                            1.0,
                            -1.0,
                            op0=mybir.AluOpType.min,
                            op1=mybir.AluOpType.max,
                        ).then_inc(cl_sem, 1)
                    if is_boundary(u):
                        nxt = (u + 1) % 2
                        if not (no_comm or barrier_only or fire_forget):
                            ve.wait_ge(rs0[par], rs_through(u))
                            ve.wait_ge(rs1[par], rs_through(u))
                        ve.memset(
                            bass.AP(F_sb[nxt], 0, [[FP, 128], [NC, KCH], [1, 1]]), 0
                        ).then_inc(cl_sem, 1)
                        ve.tensor_copy(
                            bass.AP(F_sb[nxt], 1, [[FP, 128], [NC, KCH], [1, NC - 1]]),
                            bass.AP(Fend, 0, [[FP, 128], [NC, KCH], [1, NC - 1]]),
                        ).then_inc(cl_sem, 1)
                # readout copies
                j = 0
                for mo in range(4):
                    for nt in range(4):
                        ve.wait_ge(pe_sem, ro_mm_cnt[j])
                        ve.tensor_copy(
                            outsb[:, T * mo + 512 * nt : T * mo + 512 * (nt + 1)],
                            pRO[j % 2][:, :],
                        ).then_inc(ro_sem, 1)
                        j += 1

            @block.scalar
            def _(sc):
                for u in range(NS):
                    # hist has one slot per superstep: no reuse, no DMA
                    # release wait needed before overwriting.
                    for m in range(4):
                        sc.wait_ge(pe_sem, tr_cnt[u][m])
                        sc.activation(
                            hist[:, SHARD * u + 128 * m : SHARD * u + 128 * (m + 1)],
                            pT[m % 2][:, :],
                            mybir.ActivationFunctionType.Copy,
                        ).then_inc(cp_sem, 1)

            @block.gpsimd
            def _(gp):
                if no_comm:
                    return
                gp.load_library(library_config.remote_dma)
                gp.collective_compute(
                    "AllReduce",
                    mybir.AluOpType.add,
                    replica_groups=[list(range(N_CORES))],
                    ins=[bar_in.ap().opt()],
                    outs=[bar_out.ap().opt()],
                ).then_inc(bar_sem, 1)
                gp.wait_ge(bar_sem, 1)

                cid = gp.partition_id()
                with gp.register("r0") as R0, gp.register("r1") as R1:
                    gp.reg_alu(R0, cid, SHARD, op=mybir.AluOpType.mult)
                    gp.reg_alu(R1, R0, 256, op=mybir.AluOpType.add)
                    c0 = gp.snap(R0, min_val=0, max_val=SHARD * 7)
                    c1 = gp.snap(R1, min_val=256, max_val=SHARD * 7 + 256)

                    def prep(u):
                        par = u % 2
                        dst = Fend if is_boundary(u) else F_sb[(u + 1) % 2]
                        gp.remote_dma_broadcast(
                            dst[:, bass.ds(c0, 256)],
                            hist[:, SHARD * u : SHARD * u + 256],
                            remote_sem=rs0[par],
                            local_sem=ls_sem,
                            rdests=[(0, k) for k in range(N_CORES)],
                        ).then_inc(prep_sem, 1)
                        gp.remote_dma_broadcast(
                            dst[:, bass.ds(c1, 256)],
                            hist[:, SHARD * u + 256 : SHARD * u + 512],
                            remote_sem=rs1[par],
                            local_sem=ls_sem,
                            rdests=[(0, k) for k in range(N_CORES)],
                        ).then_inc(prep_sem, 1)

                    if not barrier_only:
                        prep(0)
                    for u in range(NS):
                        if barrier_only or not sends[u]:
                            continue
                        gp.wait_ge(prep_sem, 2 * u + 1)
                        gp.wait_ge(cp_sem, copy_cnt[u][1])
                        gp.trigger_dma(count=1)
                        gp.wait_ge(prep_sem, 2 * u + 2)
                        gp.wait_ge(cp_sem, copy_cnt[u][3])
                        gp.trigger_dma(count=1)
                        if u + 1 < NS and sends[u + 1]:
                            prep(u + 1)
                    if not barrier_only:
                        gp.wait_ge(ls_sem, ls_total)
                gp.collective_compute(
                    "AllReduce",
                    mybir.AluOpType.add,
                    replica_groups=[list(range(N_CORES))],
                    ins=[bar_in.ap().opt()],
                    outs=[bar_out.ap().opt()],
                ).then_inc(bar_sem, 1)
                gp.wait_ge(bar_sem, 2)

    mybir.codegen_inst_isa_subclasses(nc)
    return nc


# ---------------- host-side data prep ----------------

def prep_inputs(W, x, w_out, T):
    """Returns per-core input dicts. Arg order matches the old kernel."""
    W = np.asarray(W, np.float32)
    x = np.asarray(x, np.float32)
    w_out = np.asarray(w_out, np.float32)
    L = T // NC
    WT = np.ascontiguousarray(W.T)                      # [feat_in, row]
    maps = []
    for c in range(N_CORES):
        lo = SHARD * c
        # wT[p, k*512+r] = W[lo+r, 128k+p]
        wT = np.ascontiguousarray(
            WT.reshape(KCH, 128, FEAT)[:, :, lo : lo + SHARD]
            .transpose(1, 0, 2)
            .reshape(128, KCH * SHARD)
        ).astype(np.float16)
        # xT[g, i*512+r] = x[lo+r, L*g+i]
        xT = np.ascontiguousarray(
            x[lo : lo + SHARD, :].reshape(SHARD, NC, L)
            .transpose(1, 2, 0)
            .reshape(NC, L * SHARD)
        ).astype(np.float16)
        # woT[p, k*512 + mo*128 + o] = w_out[128mo+o, lo + 128k + p]
        woT = np.ascontiguousarray(
            w_out[:, lo : lo + SHARD].reshape(4, 128, 4, 128)
            .transpose(3, 2, 0, 1)
            .reshape(128, 4 * OUT_DIM)
        ).astype(np.float16)
        maps.append({"wT": wT, "xT": xT, "woT": woT})
    return maps


from concourse.bass_utils import run_bass_kernel_spmd

_cache = {}


def kernel(x: np.ndarray, W_res: np.ndarray, w_out: np.ndarray) -> np.ndarray:
    T = x.shape[1]
    if "nc" not in _cache or _cache.get("T") != T:
        _cache["nc"] = build(T)
        _cache["T"] = T
    nc = _cache["nc"]
    maps = prep_inputs(np.asarray(W_res), np.asarray(x), np.asarray(w_out), T)
    res = run_bass_kernel_spmd(nc, maps, core_ids=list(range(N_CORES)))
    raw = np.zeros((OUT_DIM, T), np.float32)
    for r in res.results:
        raw += r["outp"]
    # cols are [nt, ii, gg] with t = gg*L + nt*(L//4) + ii; invert
    L = T // NC
    out = raw.reshape(OUT_DIM, 4, L // 4, NC).transpose(0, 3, 1, 2).reshape(OUT_DIM, T)
    return out
